# revision 1
# baseline (speedup 1.0000x reference)
"""BatchedACE (soft clustered linear attention) Trainium2 kernel.

Full inputs -> full output. Sharding: N = M*B*H batch axis across 8 cores;
core c handles (m, b) = (c//2, c%2), i.e. all 8 heads of one (ensemble,
batch) pair, whose K/Q/V slices are contiguous 8 MiB blocks of HBM.

Per (m, b): for each head h and T-tile of 128 rows:
  projK/Q = K/Q @ planes (contract d=64, via PE transpose + block-diag planes)
  logits  = tanh(proj) @ blockdiag(protos/scale)    (contract 32)
  probsK  = softmax16(logitsK)   (T-major: DVE group-reduce + recip + mult)
  probsQT = softmax16(logitsQT)  (S-major: PE group-sum + PE broadcast matmul)
  b_sum/A accumulate in PSUM via probsK.T @ [V|1]
  E = b_sum / (A + eps); out = probsQT.T @ E  (phase 2, probsQT stashed bf16)
"""

import itertools

import numpy as np
import ml_dtypes

import concourse.bacc as bacc
import concourse.mybir as mybir
import concourse.tile as tile

F32 = mybir.dt.float32
BF16 = mybir.dt.bfloat16
AF = mybir.ActivationFunctionType
MULT = mybir.AluOpType.mult

D_K, K_BITS, L_TAB, M_ENS = 64, 4, 8, 4
R = 1 << K_BITS          # 16
S = L_TAB * R            # 128
B, T, H = 2, 4096, 8
EPS = 1e-06
HD = H * D_K             # 512
TT = 128                 # T tile rows
NT = T // TT             # 32 tiles


def _build_module():
    nc = bacc.Bacc("TRN2", target_bir_lowering=False, debug=False,
                   num_devices=8, enable_asserts=False)

    K = nc.dram_tensor("K", [T, HD], F32, kind="ExternalInput")
    Q = nc.dram_tensor("Q", [T, HD], F32, kind="ExternalInput")
    V = nc.dram_tensor("V", [T, HD], F32, kind="ExternalInput")
    planes2e = nc.dram_tensor("planes2e", [128, 32], F32, kind="ExternalInput")
    planes2o = nc.dram_tensor("planes2o", [128, 32], F32, kind="ExternalInput")
    blockdiag = nc.dram_tensor("blockdiag", [32, 128], BF16, kind="ExternalInput")
    identity = nc.dram_tensor("identity", [128, 128], F32, kind="ExternalInput")
    ind = nc.dram_tensor("ind", [128, 8], BF16, kind="ExternalInput")
    indT = nc.dram_tensor("indT", [8, 128], F32, kind="ExternalInput")
    O = nc.dram_tensor("O", [H, T, D_K], F32, kind="ExternalOutput")

    with tile.TileContext(nc) as tc:
        with (
            tc.tile_pool(name="pconst", bufs=1) as pconst,
            tc.tile_pool(name="pstash", bufs=1) as pstash,
            tc.tile_pool(name="pin", bufs=3) as pin,
            tc.tile_pool(name="pmid", bufs=2) as pmid,
            tc.tile_pool(name="pout", bufs=3) as pout,
            tc.tile_pool(name="psmall", bufs=4) as psmall,
            tc.tile_pool(name="pacc", bufs=1, space="PSUM") as pacc,
            tc.tile_pool(name="pwork", bufs=3, space="PSUM") as pwork,
        ):
            # ---- constants to SBUF
            planes2e_sb = pconst.tile([128, 32], F32)
            nc.gpsimd.dma_start(planes2e_sb[:], planes2e[:])
            planes2o_sb = pconst.tile([128, 32], F32)
            nc.gpsimd.dma_start(planes2o_sb[:], planes2o[:])
            blockdiag_sb = pconst.tile([32, 128], BF16)
            nc.gpsimd.dma_start(blockdiag_sb[:], blockdiag[:])
            identity_sb = pconst.tile([128, 128], F32)
            nc.gpsimd.dma_start(identity_sb[:], identity[:])
            ind_sb = pconst.tile([128, 8], BF16)
            nc.gpsimd.dma_start(ind_sb[:], ind[:])
            indT_sb = pconst.tile([8, 128], F32)
            nc.gpsimd.dma_start(indT_sb[:], indT[:])
            zrow = pconst.tile([1, 512], F32)
            nc.gpsimd.memset(zrow[:], 0.0)
            zcol = pconst.tile([1, 128], F32)
            nc.gpsimd.memset(zcol[:], 0.0)

            # ---- persistent PSUM accumulators: 4 heads per bank
            accA = pacc.tile([128, 4 * 65], F32)
            accB = pacc.tile([128, 4 * 65], F32)
            # zero-fill via a K=1 matmul of zeros: sets has_written for the
            # whole bank so every real b_sum matmul can accumulate
            # (start=False) in any order.
            nc.tensor.matmul(accA[:, 0:260], zcol[:], zrow[:, 0:260],
                             start=True, stop=False, skip_group_check=True)
            nc.tensor.matmul(accB[:, 0:260], zcol[:], zrow[:, 0:260],
                             start=True, stop=False, skip_group_check=True)

            # probsQ^T stash: (s, h, tile, t) bf16
            stash = pstash.tile([128, H, NT, TT], BF16)

            # ================= phase 1 =================
            for ti in range(NT):
                rows = slice(ti * TT, (ti + 1) * TT)

                tK = pin.tile([128, HD], F32, tag="tk")
                nc.gpsimd.dma_start(tK[:], K[rows, :])
                tQ = pin.tile([128, HD], F32, tag="tq")
                nc.gpsimd.dma_start(tQ[:], Q[rows, :])
                tV = pin.tile([128, H, 65], BF16, tag="tv")
                nc.gpsimd.memset(tV[:, :, 64:65], 1.0)
                nc.gpsimd.dma_start(
                    tV[:, :, 0:64],
                    V[rows, :].rearrange("t (h d) -> t h d", h=H))

                # --- transpose K, Q tiles: (t, [2h|d]) -> ([d|2h], t)
                kqT_K = pwork.tile([128, 512], F32, tag="work")
                for p in range(4):
                    nc.tensor.transpose(kqT_K[:, p * 128:(p + 1) * 128],
                                        tK[:, p * 128:(p + 1) * 128],
                                        identity_sb[:])
                kT_sb = pmid.tile([128, 512], F32, tag="ktsb")
                nc.vector.tensor_copy(kT_sb[:], kqT_K[:])

                kqT_Q = pwork.tile([128, 512], F32, tag="work")
                for p in range(4):
                    nc.tensor.transpose(kqT_Q[:, p * 128:(p + 1) * 128],
                                        tQ[:, p * 128:(p + 1) * 128],
                                        identity_sb[:])
                qT_sb = pmid.tile([128, 512], F32, tag="qtsb")
                nc.scalar.copy(qT_sb[:], kqT_Q[:])

                # --- proj (contract d=64; even/odd heads in separate
                # matmuls so every later matmul operand is base-partition 0)
                projK = pwork.tile([32, 1024], F32, tag="work")
                nc.tensor.matmul(projK[:, 0:512], planes2e_sb[:], kT_sb[:],
                                 start=True, stop=True)
                nc.tensor.matmul(projK[:, 512:1024], planes2o_sb[:], kT_sb[:],
                                 start=True, stop=True)
                tanhK = pmid.tile([32, 1024], BF16, tag="thk")
                nc.scalar.activation(tanhK[:], projK[:], AF.Tanh)

                projQ = pwork.tile([32, 1024], F32, tag="work")
                nc.tensor.matmul(projQ[:, 0:512], planes2e_sb[:], qT_sb[:],
                                 start=True, stop=True)
                nc.tensor.matmul(projQ[:, 512:1024], planes2o_sb[:], qT_sb[:],
                                 start=True, stop=True)
                tanhQ = pmid.tile([32, 1024], BF16, tag="thq")
                nc.scalar.activation(tanhQ[:], projQ[:], AF.Tanh)

                # --- K side: logits (t, s), softmax over 16-groups on DVE
                logitsK = pwork.tile([128, 1024], F32, tag="work")
                for h in range(H):
                    p, r = h // 2, h % 2
                    nc.tensor.matmul(
                        logitsK[:, h * 128:(h + 1) * 128],
                        tanhK[:, 512 * r + 128 * p:512 * r + 128 * p + 128],
                        blockdiag_sb[:], start=True, stop=True)
                expK = pmid.tile([128, 1024], BF16, tag="expk")
                nc.scalar.activation(expK[:], logitsK[:], AF.Exp)

                denomK = pmid.tile([128, 64], F32, tag="dk")
                nc.vector.reduce_sum(
                    denomK[:],
                    expK[:].rearrange("p (h l r) -> p h l r", h=H, l=L_TAB),
                    axis=mybir.AxisListType.X)
                recipK = pmid.tile([128, 64], F32, tag="rk")
                nc.vector.reciprocal_approx_fast(recipK[:], denomK[:])
                probsK = pmid.tile([128, 1024], BF16, tag="pk")
                nc.vector.tensor_tensor(
                    probsK[:].rearrange("p (h l r) -> p h l r", h=H, l=L_TAB),
                    expK[:].rearrange("p (h l r) -> p h l r", h=H, l=L_TAB),
                    recipK[:].rearrange("p (h l) -> p h l", h=H)
                        .broadcast_to((128, H, L_TAB, R)),
                    op=MULT)

                # --- b_sum / A accumulate
                for h in range(H):
                    acc = accA if h < 4 else accB
                    off = (h % 4) * 65
                    nc.tensor.matmul(
                        acc[:, off:off + 65],
                        probsK[:, h * 128:(h + 1) * 128],
                        tV[:, h, :],
                        start=False, stop=(ti == NT - 1 and h % 4 == 3),
                        skip_group_check=True)

                # --- Q side: logits (s, t), softmax via PE gsum + bcast
                logitsQT = pwork.tile([128, 1024], F32, tag="work")
                for h in range(H):
                    p, r = h // 2, h % 2
                    nc.tensor.matmul(
                        logitsQT[:, h * 128:(h + 1) * 128],
                        blockdiag_sb[:],
                        tanhQ[:, 512 * r + 128 * p:512 * r + 128 * p + 128],
                        start=True, stop=True)
                expQT = pmid.tile([128, 1024], BF16, tag="expq")
                nc.scalar.activation(expQT[:], logitsQT[:], AF.Exp)

                gsumQ = pwork.tile([8, 1024], F32, tag="work")
                for h in range(H):
                    nc.tensor.matmul(
                        gsumQ[:, h * 128:(h + 1) * 128],
                        ind_sb[:], expQT[:, h * 128:(h + 1) * 128],
                        start=True, stop=True)
                recipQS = pmid.tile([8, 1024], F32, tag="rqs")
                nc.vector.reciprocal_approx_fast(recipQS[:], gsumQ[:])

                recipQb = pwork.tile([128, 1024], F32, tag="work")
                for h in range(H):
                    nc.tensor.matmul(
                        recipQb[:, h * 128:(h + 1) * 128],
                        indT_sb[:], recipQS[:, h * 128:(h + 1) * 128],
                        start=True, stop=True)
                nc.vector.tensor_tensor(
                    stash[:, :, ti, :],
                    expQT[:].rearrange("p (h t) -> p h t", h=H),
                    recipQb[:].rearrange("p (h t) -> p h t", h=H),
                    op=MULT)

            # ================= E = b_sum / (A + eps) =================
            e_tiles = []
            for h in range(H):
                acc = accA if h < 4 else accB
                off = (h % 4) * 65
                aeps = psmall.tile([128, 1], F32, tag="ae")
                nc.vector.tensor_scalar_add(aeps[:], acc[:, off + 64:off + 65],
                                            EPS)
                recipA = psmall.tile([128, 1], F32, tag="ra")
                nc.vector.reciprocal_approx_fast(recipA[:], aeps[:])
                e_h = pconst.tile([128, 64], BF16, name=f"e_{h}")
                nc.scalar.activation(e_h[:], acc[:, off:off + 64], AF.Copy,
                                     scale=recipA[:])
                e_tiles.append(e_h)

            # ================= phase 2: out = probsQT.T @ E =================
            for ti in range(NT):
                out2 = pwork.tile([128, 512], F32, tag="work")
                for h in range(H):
                    nc.tensor.matmul(out2[:, h * 64:(h + 1) * 64],
                                     stash[:, h, ti, :], e_tiles[h][:],
                                     start=True, stop=True)
                outT = pout.tile([128, 512], F32, tag="ot")
                nc.scalar.copy(outT[:], out2[:])
                nc.gpsimd.dma_start(
                    O[:, ti * TT:(ti + 1) * TT, :].rearrange("h t d -> t h d"),
                    outT[:].rearrange("t (h d) -> t h d", h=H))

    nc.finalize()
    return nc


def _protos() -> np.ndarray:
    corners = np.array(list(itertools.product([-1.0, 1.0], repeat=K_BITS)),
                       dtype=np.float32)
    return corners.T  # (K_BITS, R)


def _consts_for(planes_m: np.ndarray, scale: float) -> dict:
    protos_s = (_protos() / scale).astype(np.float32)  # (4, 16)
    blockdiag = np.zeros((32, 128), np.float32)
    for l in range(L_TAB):
        blockdiag[l * K_BITS:(l + 1) * K_BITS, l * R:(l + 1) * R] = protos_s
    planes2e = np.zeros((128, 32), np.float32)
    planes2e[0:64, :] = planes_m
    planes2o = np.zeros((128, 32), np.float32)
    planes2o[64:128, :] = planes_m
    ind = np.zeros((128, 8), np.float32)
    for s in range(S):
        ind[s, s // R] = 1.0
    return {
        "planes2e": planes2e,
        "planes2o": planes2o,
        "blockdiag": blockdiag.astype(ml_dtypes.bfloat16),
        "identity": np.eye(128, dtype=np.float32),
        "ind": ind.astype(ml_dtypes.bfloat16),
        "indT": np.ascontiguousarray(ind.T),
    }


_NC_CACHE = None


def _get_module():
    global _NC_CACHE
    if _NC_CACHE is None:
        _NC_CACHE = _build_module()
    return _NC_CACHE


def make_in_maps(Khf, Vhf, Qhf, planes_T, logit_temp):
    Khf = np.asarray(Khf, np.float32)
    Vhf = np.asarray(Vhf, np.float32)
    Qhf = np.asarray(Qhf, np.float32)
    planes_T = np.asarray(planes_T, np.float32)
    scale = float(np.clip(np.exp(float(np.asarray(logit_temp))), 0.01, 20.0))
    in_maps = []
    for c in range(8):
        m, b = c // 2, c % 2
        consts = _consts_for(planes_T[m], scale)
        in_maps.append({
            "K": np.ascontiguousarray(Khf[m, b].reshape(T, HD)),
            "Q": np.ascontiguousarray(Qhf[m, b].reshape(T, HD)),
            "V": np.ascontiguousarray(Vhf[m, b].reshape(T, HD)),
            **consts,
        })
    return in_maps


def assemble_output(results) -> np.ndarray:
    out = np.empty((M_ENS, B, H, T, D_K), np.float32)
    for c in range(8):
        out[c // 2, c % 2] = results[c]["O"]
    return out


def kernel(Khf, Vhf, Qhf, planes_T, logit_temp) -> np.ndarray:
    from concourse.bass_utils import run_bass_kernel_spmd
    nc = _get_module()
    in_maps = make_in_maps(Khf, Vhf, Qhf, planes_T, logit_temp)
    res = run_bass_kernel_spmd(nc, in_maps, list(range(8)))
    return assemble_output(res.results)



# revision 22
# speedup vs baseline: 6.9929x; 6.9929x over previous
"""BatchedACE (soft clustered linear attention) Trainium2 kernel, v2b.

Full inputs -> full output. Sharding: N = M*B*H batch axis across 8 cores;
core c handles (m, b) = (c//2, c%2), i.e. all 8 heads of one (ensemble,
batch) pair, whose K/Q/V slices are contiguous 8 MiB blocks of HBM.

Per (m, b), tiled over T in 128-row tiles:
  K/Q loaded as bf16 (cast DMA); 8 bf16 PE transposes into one PSUM tile;
  one DVE 2x copy to SBUF.  proj = f32r-stationary x bf16-moving matmuls
  packed K rows 0:64 / Q rows 64:128 -> one tanh.
  logitsK: 8 bf16 matmuls (stationary tanh slices at 32-aligned bases).
  logitsQT: 2 merged bf16 matmuls.  softmaxK: Act exp -> DVE reduce ->
  DVE recip -> Pool pair-dup -> Pool 2-byte multiply.
  softmaxQ: PE group-sums packed 4 tiles deep into one PSUM tile (out
  partition bases 0/32/64/96), ONE DVE reciprocal per 4 tiles (f32r out),
  PE f32r broadcast, DVE multiply into the bf16 stash.
  b_sum/A accumulate in PSUM, deferred 2 tiles to hide the softmax chain.
  phase 2: E = b_sum/(A+eps); out = probsQT.T @ E from the bf16 stash.
"""

import itertools
import os

import numpy as np
import ml_dtypes

import concourse.bacc as bacc
import concourse.mybir as mybir
import concourse.tile as tile
from concourse.dve_ops import RECIP_APPROX_FAST_CONSTS, RECIPROCAL_APPROX_FAST

F32 = mybir.dt.float32
F32R = mybir.dt.float32r
BF16 = mybir.dt.bfloat16
AF = mybir.ActivationFunctionType
MULT = mybir.AluOpType.mult

D_K, K_BITS, L_TAB, M_ENS = 64, 4, 8, 4
R = 1 << K_BITS          # 16
S = L_TAB * R            # 128
B, T, H = 2, 4096, 8
EPS = 1e-06
HD = H * D_K             # 512
TT = 128                 # T tile rows
NT = T // TT             # 32 tiles
G4 = 4                   # tiles per packed Q-recip group


def _build_module(repeat=1, nt_count=NT):
    nc = bacc.Bacc("TRN2", target_bir_lowering=False, debug=False,
                   num_devices=8, enable_asserts=False)

    K = nc.dram_tensor("K", [T, HD], F32, kind="ExternalInput")
    Q = nc.dram_tensor("Q", [T, HD], F32, kind="ExternalInput")
    V = nc.dram_tensor("V", [T, HD], F32, kind="ExternalInput")
    Wp = nc.dram_tensor("Wp", [128, 64], BF16, kind="ExternalInput")
    blockdiag = nc.dram_tensor("blockdiag", [32, 128], BF16,
                               kind="ExternalInput")
    identity = nc.dram_tensor("identity", [128, 128], BF16,
                              kind="ExternalInput")
    ind = nc.dram_tensor("ind", [128, 8], BF16, kind="ExternalInput")
    indTb = nc.dram_tensor("indTb", [8, 128], BF16, kind="ExternalInput")
    O = nc.dram_tensor("O", [T, HD], F32, kind="ExternalOutput")

    with tile.TileContext(nc) as tc:
        with (
            tc.tile_pool(name="pconst", bufs=1) as pconst,
            tc.tile_pool(name="pstash", bufs=1) as pstash,
            tc.tile_pool(name="pin", bufs=3) as pin,
            tc.tile_pool(name="pmid", bufs=2) as pmid,
            tc.tile_pool(name="pout", bufs=3) as pout,
            tc.tile_pool(name="psmall", bufs=4) as psmall,
            tc.tile_pool(name="pacc", bufs=1, space="PSUM") as pacc,
            tc.tile_pool(name="pgsum", bufs=1, space="PSUM") as pgsum,
            tc.tile_pool(name="pwork", bufs=4, space="PSUM") as pwork,
        ):
            # ---- constants to SBUF
            W_sb = pconst.tile([128, 64], BF16)
            nc.sync.dma_start(W_sb[:], Wp[:])
            blockdiag_sb = pconst.tile([32, 128], BF16)
            nc.sync.dma_start(blockdiag_sb[:], blockdiag[:])
            identity_sb = pconst.tile([128, 128], BF16)
            nc.sync.dma_start(identity_sb[:], identity[:])
            ind_sb = pconst.tile([128, 8], BF16)
            nc.sync.dma_start(ind_sb[:], ind[:])
            indTb_sb = pconst.tile([8, 128], BF16)
            nc.sync.dma_start(indTb_sb[:], indTb[:])
            zrow = pconst.tile([1, 512], F32)
            nc.gpsimd.memset(zrow[:], 0.0)
            zcol = pconst.tile([1, 128], F32)
            nc.gpsimd.memset(zcol[:], 0.0)
            ones_c = pconst.tile([128, 1], BF16)
            nc.gpsimd.memset(ones_c[:], 1.0)

            # raw expQ^T stash: (s, block, tile, t) bf16; b = 4*(h%2)+h//2
            stash = pstash.tile([128, 8, NT, TT], BF16)
            # packed Q-softmax reciprocals, one [128,1024] slab per group
            rqsAll = pstash.tile([128, NT // G4, 1024], F32)

            for _rep in range(repeat):
                _emit_iteration(nc, pconst, pin, pmid, pout, psmall, pacc,
                                pgsum, pwork, stash, rqsAll, K, Q, V, O, W_sb,
                                blockdiag_sb, identity_sb, ind_sb, indTb_sb,
                                zrow, zcol, ones_c, nt_count)

    nc.finalize()
    return nc


def _emit_iteration(nc, pconst, pin, pmid, pout, psmall, pacc, pgsum, pwork,
                    stash, rqsAll, K, Q, V, O, W_sb, blockdiag_sb, identity_sb,
                    ind_sb, indTb_sb, zrow, zcol, ones_c, nt_count=NT):
    # persistent PSUM accumulators: 4 heads per bank, cols h%4*65 (+64 = A)
    accA = pacc.tile([128, 4 * 65], F32, tag="accA")
    accB = pacc.tile([128, 4 * 65], F32, tag="accB")
    # zero-fill via a K=1 matmul of zeros: sets has_written for the whole
    # bank so every real b_sum matmul can accumulate (start=False).
    nc.tensor.matmul(accA[:, 0:260], zcol[:], zrow[:, 0:260],
                     start=True, stop=False, skip_group_check=True)
    nc.tensor.matmul(accB[:, 0:260], zcol[:], zrow[:, 0:260],
                     start=True, stop=False, skip_group_check=True)

    def load_tile(ti):
        rows = slice(ti * TT, (ti + 1) * TT)
        tK = pin.tile([128, HD], BF16, tag="tk", bufs=3)
        nc.gpsimd.dma_start(tK[:], K[rows, :])
        tQ = pin.tile([128, HD], BF16, tag="tq", bufs=3)
        nc.gpsimd.dma_start(tQ[:], Q[rows, :])
        tV = pin.tile([128, HD], BF16, tag="tv", bufs=4)
        nc.gpsimd.dma_start(tV[:], V[rows, :])
        return tK, tQ, tV

    def emit_bsum(st):
        """b_sum/A accumulation for state dict st (deferred 2 tiles)."""
        probsK, tV = st["probsK"], st["tV"]
        last = st["ti"] == st["nt"] - 1
        for h in range(H):
            acc = accA if h < 4 else accB
            off = (h % 4) * 65
            nc.tensor.matmul(
                acc[:, off:off + 64],
                probsK[:, h * 128:(h + 1) * 128],
                tV[:, h * 64:(h + 1) * 64],
                start=False, stop=False,
                skip_group_check=True)
            nc.tensor.matmul(
                acc[:, off + 64:off + 65],
                probsK[:, h * 128:(h + 1) * 128],
                ones_c[:],
                start=False, stop=(last and h % 4 == 3),
                skip_group_check=True)

    # --- Q-side softmax state, packed per group of G4 tiles ---
    qgroup = {"gsum_e": None, "gsum_o": None}
    S = {}               # per-tile state
    pending_tail = []    # states awaiting bcast+stash (deferred per group)

    def emit_gsum(ti):
        """Group-sums of expQT(ti) into the packed gsum tiles at
        partition base 32*(ti%4); one packed recip + row shifts per group."""
        st = S[ti]
        k = ti % G4
        if k == 0:
            qgroup["gsum_e"] = pgsum.tile([128, 512], F32, tag="ge")
            qgroup["gsum_o"] = pgsum.tile([128, 512], F32, tag="go")
        ge, go = qgroup["gsum_e"], qgroup["gsum_o"]
        last = (k == G4 - 1) or (ti == st["nt"] - 1)
        expQT = st["expQT"]
        nc.tensor.matmul(ge[32 * k:32 * k + 8, :], ind_sb[:],
                         expQT[:, 0:512], start=True, stop=True,
                         skip_group_check=True, tile_position=(0, 32 * k))
        nc.tensor.matmul(go[32 * k:32 * k + 8, :], ind_sb[:],
                         expQT[:, 512:1024], start=True, stop=True,
                         skip_group_check=True, tile_position=(0, 32 * k))
        pending_tail.append(st)
        if last:
            recipQS = pmid.tile([128, 1024], F32R, tag="rqs", bufs=2)
            c = RECIP_APPROX_FAST_CONSTS
            nc.vector._custom_dve(RECIPROCAL_APPROX_FAST,
                                  out=recipQS[:, 0:512], in0=ge[:],
                                  s0=c["s0"], s1=c["s1"], imm2=c["imm2"])
            nc.vector._custom_dve(RECIPROCAL_APPROX_FAST,
                                  out=recipQS[:, 512:1024], in0=go[:],
                                  s0=c["s0"], s1=c["s1"], imm2=c["imm2"])
            # shift each tile's 8 rows down to partition base 0 (matmul
            # operands with row base != 0 are unsafe on HW)
            rqs4 = pmid.tile([8, G4, 1024], F32R, tag="rqs4", bufs=3)
            for kk in range(G4):
                nc.sync.dma_start(rqs4[0:8, kk, :],
                                  recipQS[32 * kk:32 * kk + 8, :])
            for s in pending_tail:
                s.setdefault("rqs4", rqs4)

    def emit_qtail(st):
        """Broadcast + stash multiply for tile st (after its group recip)."""
        expQT, rqs4, ti = st["expQT"], st["rqs4"], st["ti"]
        k = ti % G4
        rQbe = pwork.tile([128, 512], F32, tag="work", name="rQbe")
        nc.tensor.matmul(rQbe[:], indT_sb[0:8, :], rqs4[0:8, k, 0:512],
                         start=True, stop=True)
        rQbo = pwork.tile([128, 512], F32, tag="work", name="rQbo")
        nc.tensor.matmul(rQbo[:], indT_sb[0:8, :], rqs4[0:8, k, 512:1024],
                         start=True, stop=True)
        nc.vector.tensor_tensor(
            stash[:, 0:4, ti, :],
            expQT[:, 0:512].rearrange("p (b t) -> p b t", b=4),
            rQbe[:].rearrange("p (b t) -> p b t", b=4),
            op=MULT)
        nc.vector.tensor_tensor(
            stash[:, 4:8, ti, :],
            expQT[:, 512:1024].rearrange("p (b t) -> p b t", b=4),
            rQbo[:].rearrange("p (b t) -> p b t", b=4),
            op=MULT)

    def stage_transpose(ti):
        st = S[ti]
        tK, tQ = st["tK"], st["tQ"]
        kqT = pwork.tile([128, 1024], BF16, tag="work", name="kqT")
        for p in range(4):
            nc.tensor.transpose(kqT[:, p * 128:(p + 1) * 128],
                                tK[:, p * 128:(p + 1) * 128],
                                identity_sb[:])
        for p in range(4):
            nc.tensor.transpose(kqT[:, 512 + p * 128:512 + (p + 1) * 128],
                                tQ[:, p * 128:(p + 1) * 128],
                                identity_sb[:])
        kqT_sb = pmid.tile([128, 1024], BF16, tag="kqtsb", bufs=2)
        nc.vector.tensor_copy(kqT_sb[:], kqT[:])
        st["kqT_sb"] = kqT_sb

    def stage_proj(ti):
        st = S[ti]
        kqT_sb = st["kqT_sb"]
        projKQ = pwork.tile([128, 512], F32, tag="work", name="projKQ")
        nc.tensor.matmul(projKQ[0:64, :], W_sb[:], kqT_sb[:, 0:512],
                         start=True, stop=True, tile_position=(0, 0))
        nc.tensor.matmul(projKQ[64:128, :], W_sb[:], kqT_sb[:, 512:1024],
                         start=True, stop=True, tile_position=(0, 64))
        st["projKQ"] = projKQ

    def stage_tanh(ti):
        st = S[ti]
        tanhKQ = pmid.tile([128, 512], BF16, tag="thkq", bufs=3)
        nc.scalar.activation(tanhKQ[:], st["projKQ"], AF.Tanh)
        # shift K-odd / Q-even / Q-odd tanh rows to partition base 0
        tanhKo = pmid.tile([32, 512], BF16, tag="thko", bufs=3)
        nc.sync.dma_start(tanhKo[0:32, :], tanhKQ[32:64, :])
        tanhQe = pmid.tile([32, 512], BF16, tag="thqe", bufs=3)
        nc.sync.dma_start(tanhQe[0:32, :], tanhKQ[64:96, :])
        tanhQo = pmid.tile([32, 512], BF16, tag="thqo", bufs=3)
        nc.sync.dma_start(tanhQo[0:32, :], tanhKQ[96:128, :])
        st.update(tanhKQ=tanhKQ, tanhKo=tanhKo, tanhQe=tanhQe, tanhQo=tanhQo)

    def stage_logitsK(ti):
        st = S[ti]
        lKa = pwork.tile([128, 512], F32, tag="work", name="lKa")
        lKb = pwork.tile([128, 512], F32, tag="work", name="lKb")
        for h in range(H):
            p, r = h // 2, h % 2
            src_t = st["tanhKQ"] if r == 0 else st["tanhKo"]
            out = (lKa if h < 4 else lKb)[:, (h % 4) * 128:(h % 4 + 1) * 128]
            nc.tensor.matmul(out, src_t[0:32, 128 * p:128 * (p + 1)],
                             blockdiag_sb[:], start=True, stop=True)
        expK = pmid.tile([128, 1024], BF16, tag="expk", bufs=4)
        nc.scalar.activation(expK[:, 0:512], lKa[:], AF.Exp)
        nc.scalar.activation(expK[:, 512:1024], lKb[:], AF.Exp)
        st["expK"] = expK

    def stage_logitsQ(ti):
        st = S[ti]
        lQTe = pwork.tile([128, 512], F32, tag="work", name="lQTe")
        nc.tensor.matmul(lQTe[:], blockdiag_sb[:], st["tanhQe"][0:32, :],
                         start=True, stop=True)
        lQTo = pwork.tile([128, 512], F32, tag="work", name="lQTo")
        nc.tensor.matmul(lQTo[:], blockdiag_sb[:], st["tanhQo"][0:32, :],
                         start=True, stop=True)
        ti = st["ti"]
        nc.scalar.activation(
            stash[:, 0:4, ti, :],
            lQTe[:].rearrange("p (b t) -> p b t", b=4), AF.Exp)
        nc.scalar.activation(
            stash[:, 4:8, ti, :],
            lQTo[:].rearrange("p (b t) -> p b t", b=4), AF.Exp)

    def stage_reduceK(ti):
        st = S[ti]
        expK = st["expK"]
        denomK = psmall.tile([128, 64], F32, tag="dk", bufs=3)
        nc.vector.reduce_sum(
            denomK[:],
            expK[:].rearrange("p (h l r) -> p h l r", h=H, l=L_TAB),
            axis=mybir.AxisListType.X)
        recipK = psmall.tile([128, 64], F32, tag="rk", bufs=3)
        nc.vector.reciprocal_approx_fast(recipK[:], denomK[:])
        st["recipK"] = recipK

    def stage_probsK(ti):
        st = S[ti]
        expK, recipK = st["expK"], st["recipK"]
        recipK2 = psmall.tile([128, 128], BF16, tag="rk2", bufs=3)
        nc.gpsimd.tensor_copy(
            recipK2[:].rearrange("p (hl j) -> p hl j", hl=H * L_TAB),
            recipK[:].unsqueeze(2).broadcast_to((128, H * L_TAB, 2)))
        probsK = pmid.tile([128, 1024], BF16, tag="pk", bufs=5)
        nc.gpsimd.tensor_tensor(
            probsK[:].rearrange("p (hl g j) -> p hl g j", hl=H * L_TAB,
                                g=R // 2),
            expK[:].rearrange("p (hl g j) -> p hl g j", hl=H * L_TAB,
                              g=R // 2),
            recipK2[:].rearrange("p (hl j) -> p hl j", hl=H * L_TAB)
                .unsqueeze(2).broadcast_to((128, H * L_TAB, R // 2, 2)),
            op=MULT)
        st["probsK"] = probsK

    # ================= phase 1 (software-pipelined) =================
    S[0] = dict(ti=0, nt=nt_count)
    S[0]["tK"], S[0]["tQ"], S[0]["tV"] = load_tile(0)

    for it in range(nt_count + 3):
        def live(t):
            return 0 <= t < nt_count
        if live(it + 1):
            S[it + 1] = dict(ti=it + 1, nt=nt_count)
            (S[it + 1]["tK"], S[it + 1]["tQ"],
             S[it + 1]["tV"]) = load_tile(it + 1)
        if live(it):
            stage_transpose(it)
        if live(it - 2):
            emit_gsum(it - 2)
        if live(it - 3):
            emit_bsum(S[it - 3])
        if live(it):
            stage_proj(it)
        if live(it - 1):
            stage_logitsQ(it - 1)
            stage_logitsK(it - 1)
        if live(it):
            stage_tanh(it)
        if live(it - 1):
            stage_reduceK(it - 1)
        if live(it - 2):
            stage_probsK(it - 2)
        # free old state
        S.pop(it - 6, None)

    # ================= E = b_sum / (A + eps) =================
    e_tiles = []
    for h in range(H):
        acc = accA if h < 4 else accB
        off = (h % 4) * 65
        aeps = psmall.tile([128, 1], F32, tag="ae")
        nc.vector.tensor_scalar_add(aeps[:], acc[:, off + 64:off + 65], EPS)
        recipA = psmall.tile([128, 1], F32, tag="ra")
        nc.vector.reciprocal_approx_fast(recipA[:], aeps[:])
        e_h = pconst.tile([128, 64], BF16, name=f"e_{h}", tag=f"e_{h}")
        nc.scalar.activation(e_h[:], acc[:, off:off + 64], AF.Copy,
                             scale=recipA[:])
        e_tiles.append(e_h)

    # ================= phase 2: out = probsQT.T @ E =================
    def shift_group(g):
        """Cast+shift group g's reciprocal rows down to partition base 0."""
        r4 = pmid.tile([8, G4, 1024], BF16, tag="rqs4b", bufs=2, name="r4")
        for kk in range(G4):
            nc.gpsimd.dma_start(r4[0:8, kk, :],
                                rqsAll[32 * kk:32 * kk + 8, g, :])
        return r4

    ngroups = (nt_count + G4 - 1) // G4
    outT = None
    rq_next = shift_group(0)
    rq_cur = None
    for ti in range(nt_count):
        g, k = divmod(ti, G4)
        if k == 0:
            rq_cur = rq_next
            if g + 1 < ngroups:
                rq_next = shift_group(g + 1)
        rQbe = pwork.tile([128, 512], F32, tag="work", name="rQbe")
        nc.tensor.matmul(rQbe[:], indTb_sb[0:8, :], rq_cur[0:8, k, 0:512],
                         start=True, stop=True)
        rQbo = pwork.tile([128, 512], F32, tag="work", name="rQbo")
        nc.tensor.matmul(rQbo[:], indTb_sb[0:8, :], rq_cur[0:8, k, 512:1024],
                         start=True, stop=True)
        pq = pmid.tile([128, 1024], BF16, tag="pq", bufs=3, name="pq")
        nc.vector.tensor_tensor(
            pq[:, 0:512].rearrange("p (b t) -> p b t", b=4),
            stash[:, 0:4, ti, :],
            rQbe[:].rearrange("p (b t) -> p b t", b=4),
            op=MULT)
        nc.vector.tensor_tensor(
            pq[:, 512:1024].rearrange("p (b t) -> p b t", b=4),
            stash[:, 4:8, ti, :],
            rQbo[:].rearrange("p (b t) -> p b t", b=4),
            op=MULT)
        out2 = pwork.tile([128, 512], F32, tag="work", name="out2")
        for h in range(H):
            p, r = h // 2, h % 2
            b = 4 * r + p
            nc.tensor.matmul(out2[:, h * 64:(h + 1) * 64],
                             pq[:, b * 128:(b + 1) * 128], e_tiles[h][:],
                             start=True, stop=True)
        if ti % 2 == 0:
            outT = pout.tile([128, 2 * 512], F32, tag="ot")
            nc.scalar.copy(outT[:, 0:512], out2[:])
        else:
            nc.scalar.copy(outT[:, 512:1024], out2[:])
            # two tiles per DMA: dst rows (2,128) viewed partition-major
            nc.sync.dma_start(
                O[(ti - 1) * TT:(ti + 1) * TT, :]
                    .rearrange("(j p) c -> p j c", j=2),
                outT[:].rearrange("p (j c) -> p j c", j=2))


def _protos() -> np.ndarray:
    corners = np.array(list(itertools.product([-1.0, 1.0], repeat=K_BITS)),
                       dtype=np.float32)
    return corners.T  # (K_BITS, R)


def _consts_for(planes_m: np.ndarray, scale: float) -> dict:
    protos_s = (_protos() / scale).astype(np.float32)  # (4, 16)
    blockdiag = np.zeros((32, 128), np.float32)
    for l in range(L_TAB):
        blockdiag[l * K_BITS:(l + 1) * K_BITS, l * R:(l + 1) * R] = protos_s

    Wp = np.zeros((128, 64), np.float32)
    Wp[0:64, 0:32] = planes_m
    Wp[64:128, 32:64] = planes_m
    ind = np.zeros((128, 8), np.float32)
    for s in range(S):
        ind[s, s // R] = 1.0
    return {
        "Wp": Wp.astype(ml_dtypes.bfloat16),
        "blockdiag": blockdiag.astype(ml_dtypes.bfloat16),
        "identity": np.eye(128, dtype=ml_dtypes.bfloat16),
        "ind": ind.astype(ml_dtypes.bfloat16),
        "indTb": np.ascontiguousarray(ind.T).astype(ml_dtypes.bfloat16),
    }


_NC_CACHE = None


def _get_module():
    global _NC_CACHE
    if _NC_CACHE is None:
        _NC_CACHE = _build_module()
    return _NC_CACHE


def make_in_maps(Khf, Vhf, Qhf, planes_T, logit_temp):
    Khf = np.asarray(Khf, np.float32)
    Vhf = np.asarray(Vhf, np.float32)
    Qhf = np.asarray(Qhf, np.float32)
    planes_T = np.asarray(planes_T, np.float32)
    scale = float(np.clip(np.exp(float(np.asarray(logit_temp))), 0.01, 20.0))
    in_maps = []
    for c in range(8):
        m, b = c // 2, c % 2
        consts = _consts_for(planes_T[m], scale)
        in_maps.append({
            "K": np.ascontiguousarray(Khf[m, b].reshape(T, HD)),
            "Q": np.ascontiguousarray(Qhf[m, b].reshape(T, HD)),
            "V": np.ascontiguousarray(Vhf[m, b].reshape(T, HD)),
            **consts,
        })
    return in_maps


def assemble_output(results) -> np.ndarray:
    out = np.empty((M_ENS, B, H, T, D_K), np.float32)
    for c in range(8):
        o = results[c]["O"].reshape(T, H, D_K)
        out[c // 2, c % 2] = o.transpose(1, 0, 2)
    return out


def kernel(Khf, Vhf, Qhf, planes_T, logit_temp) -> np.ndarray:
    from concourse.bass_utils import run_bass_kernel_spmd
    nc = _get_module()
    in_maps = make_in_maps(Khf, Vhf, Qhf, planes_T, logit_temp)
    res = run_bass_kernel_spmd(nc, in_maps, list(range(8)))
    return assemble_output(res.results)
